# revision 1
# baseline (speedup 1.0000x reference)
"""Conv2d 3x3 (stride 1, pad 1) forward, data-parallel over batch on 8 trn2 cores.

x (16,64,224,224) * w (128,64,3,3) + b (128,) -> (16,128,224,224), fp32.

Per core (2 images), compute in bf16 with fp32 PSUM accumulate:

- Rows are even/odd phase-split on the SBUF partition axis (p = phase*64 + ci).
  Host pre-packs x into a padded bf16 layout (xp) so every DMA moves >=6KB
  contiguous runs per partition (<512B chunks run at half DMA rate).
- Per PSUM tile (2 output rows x 228 = 456 wide) the 9 conv taps are covered
  by 5 matmuls (the minimum: 9 taps = 4 K=128 pairs + 1 K=64 single):
  3 "A" pairs (two kh taps at the same kw, from the xt tile), 1 "C"/"D" pair
  over a tile holding the remaining kh row at two horizontal shifts (two kw
  taps in one matmul), and 1 K=64 single. The shifted tiles are built from a
  6.5MB/img HBM re-read (the cross-partition half) plus an on-chip DVE
  column-shift copy (the same-partition half), instead of a full host-side
  duplicate.
- Matmuls are issued weight-major over pairs of j0 steps (KWMAJ): consecutive
  matmuls share the same stationary operand, halving PE weight reloads
  (measured ~17us/iter on hw). Output stores issue from the Activation
  engine's DGE queue so input loads (SP queue) are never stuck behind
  compute-dependent stores; obo bias-adds run on DVE to halve Act load.
- Outputs are written as bf16 in a phase-split layout; the host re-interleaves
  and upcasts to fp32. End-to-end rel err ~3.6e-3 (tolerance 2e-2).

Measured (1024-iter hw loop, slope method): ~310us/iter vs 557us baseline.
"""

import os
import sys

sys.path.insert(0, "/opt/trn_rl_repo")

import numpy as np
import ml_dtypes

import concourse.bass as bass
import concourse.mybir as mybir
import concourse.tile as tile
from concourse import bacc
from concourse.bass_utils import run_bass_kernel_spmd

N_FULL, C_IN, H, W = 16, 64, 224, 224
C_OUT = 128
N_CORES = 8
N_PER = N_FULL // N_CORES  # 2 images per core
HH = H // 2  # 112 row-pairs

U = int(os.environ.get("KU", "16"))  # row-pairs of output per block
SLOTS = U + 2
SW = 228  # padded row width: [pad, 224 data, pad, pad, pad]
NB = HH // U  # 8 blocks per image
F32 = mybir.dt.float32
BF16 = mybir.dt.bfloat16
NPBF16 = ml_dtypes.bfloat16

_CACHE = {}


def _build_nc(repeat=1, loop_reps=0, ablate=(), scheme=None, xcd_bufs=None,
              out_bufs=None, psum_bufs=None):
    """scheme: 'dual' = C+D pairs (10 matmuls/j0), 'pe_only' = C pair only
    (11 matmuls), 'base' = no shifted tiles (12 matmuls)."""
    ablate = set(ablate) | {a for a in os.environ.get("KABLATE", "").split(",") if a}
    scheme = scheme or os.environ.get("KSCHEME", "dual")
    xcd_bufs = xcd_bufs or int(os.environ.get("KXCD", "3"))
    out_bufs = out_bufs or int(os.environ.get("KOUT", "3"))
    psum_bufs = psum_bufs or int(os.environ.get("KPSUM", "2"))
    st_eng = os.environ.get("KSTQ", "act")
    c_src = os.environ.get("KCSRC", "hbm")
    act_mode = os.environ.get("KACT", "split")
    act_split = act_mode in ("split", "dve")
    wmaj = os.environ.get("KWMAJ", "1") == "1"
    ldq_pool = os.environ.get("KLDQ", "sp") == "pool"
    cpe_pool = os.environ.get("KCPE", "dve") == "pool"
    st_split = os.environ.get("KSTSPL", "0") == "1"
    xmrg = os.environ.get("KXMRG", "0") == "1"
    use_c = scheme in ("dual", "pe_only")
    use_d = scheme == "dual"
    nc = bacc.Bacc("TRN2", target_bir_lowering=False, debug=False)
    # Host-packed inputs (see _make_in_maps):
    #   xp[n, ph, ci, r, c]: row 2r+ph, buffer col c = x[c-1] (cols 1..224 data)
    #   xc[n, p, r, c]: p<64 odd rows shifted (col c = x[c-2]); p>=64 odd rows
    #   xd[n, p, r, c]: same for even rows
    xp_t = nc.dram_tensor("xp", [N_PER, 2, C_IN, HH, SW], BF16, kind="ExternalInput")
    wt_t = nc.dram_tensor("wt", [128, 11, 128], BF16, kind="ExternalInput")
    b_t = nc.dram_tensor("bias", [C_OUT], F32, kind="ExternalInput")
    o_t = nc.dram_tensor("out", [N_PER, 2, C_OUT, HH, 224], BF16, kind="ExternalOutput")
    xp_ap = xp_t.ap()
    wt_ap, b_ap, o_ap = wt_t.ap(), b_t.ap(), o_t.ap()

    with tile.TileContext(nc) as tc:
        with (
            tc.tile_pool(name="const", bufs=1) as cpool,
            tc.tile_pool(name="outp", bufs=out_bufs) as opool,
            tc.tile_pool(name="psum", bufs=psum_bufs, space="PSUM") as ppool,
        ):
            wsb = cpool.tile([128, 11, 128], BF16)
            nc.sync.dma_start(wsb[:], wt_ap)
            bias_sb = cpool.tile([128, 1], F32)
            nc.sync.dma_start(bias_sb[:], b_ap[:, None])

            xbufs = [cpool.tile([128, SLOTS * SW + 4], BF16, name=f"xb{i}") for i in range(xcd_bufs)]
            cbufs = [cpool.tile([128, SLOTS * SW + 4], BF16, name=f"cb{i}") for i in range(xcd_bufs)] if use_c else []
            dbufs = [cpool.tile([128, SLOTS * SW + 4], BF16, name=f"db{i}") for i in range(xcd_bufs)] if use_d else []
            for tl in xbufs + cbufs + dbufs:
                nc.vector.memset(tl[:], 0.0)

            import contextlib
            loop_cm = tc.For_i(0, loop_reps, 1) if loop_reps else contextlib.nullcontext()
            with loop_cm:
             for rep in range(repeat):
              for n in range(N_PER):
                  o_ph = o_ap[n]
                  for b in range(NB):
                      u0 = b * U
                      bi = (rep * N_PER * NB + n * NB + b) % xcd_bufs
                      xt = xbufs[bi]
                      ct = cbufs[bi] if use_c else None
                      dt_ = dbufs[bi] if use_d else None
                      xtv = xt[:, : SLOTS * SW].rearrange("p (r c) -> p r c", c=SW)

                      if b == 0:
                          nc.vector.memset(xt[:, 0:SW], 0.0)
                          if use_c:
                              nc.vector.memset(ct[:, 0:SW], 0.0)
                      if b == NB - 1:
                          nc.vector.memset(xt[:, (U + 1) * SW : (U + 2) * SW], 0.0)
                          if use_d:
                              nc.vector.memset(dt_[:, (U + 1) * SW : (U + 2) * SW], 0.0)
                      r_lo = 1 if b == 0 else 0
                      r_hi = U + 1 if b == NB - 1 else U + 2
                      pa = u0 - 1 + r_lo  # first row-pair index fetched
                      pb = u0 - 1 + r_hi  # one past last
                      if "noload" not in ablate:
                          if xmrg:
                              nc.sync.dma_start(
                                  xtv[:, r_lo:r_hi, :],
                                  xp_ap[n].rearrange("ph ci r c -> (ph ci) r c")[:, pa:pb, :],
                              )
                          else:
                              for ph in range(2):
                                  nc.sync.dma_start(
                                      xtv[ph * 64 : (ph + 1) * 64, r_lo:r_hi, :],
                                      xp_ap[n, ph, :, pa:pb, :],
                                  )
                      if use_c:
                          ctv = ct[:, : SLOTS * SW].rearrange("p (r c) -> p r c", c=SW)
                          ldeng = nc.gpsimd if ldq_pool else nc.sync
                          if c_src == "sbuf":
                              ldeng.dma_start(ctv[0:64, r_lo:r_hi, :], xtv[64:128, r_lo:r_hi, :])
                          else:
                              ldeng.dma_start(ctv[0:64, r_lo:r_hi, :], xp_ap[n, 1, :, pa:pb, :])
                          if "nocopy" not in ablate:
                              cpeng = nc.gpsimd if cpe_pool else nc.vector
                              cpeng.tensor_copy(
                                  ct[64:128, r_lo * SW + 1 : r_hi * SW],
                                  xt[64:128, r_lo * SW : r_hi * SW - 1],
                              )
                      if use_d:
                          dtv = dt_[:, : SLOTS * SW].rearrange("p (r c) -> p r c", c=SW)
                          ldeng = nc.gpsimd if ldq_pool else nc.sync
                          if c_src == "sbuf":
                              ldeng.dma_start(dtv[64:128, r_lo:r_hi, :], xtv[0:64, r_lo:r_hi, :])
                          else:
                              ldeng.dma_start(dtv[64:128, r_lo:r_hi, :], xp_ap[n, 0, :, pa:pb, :])
                          if "nocopy" not in ablate:
                              cpeng = nc.gpsimd if cpe_pool else nc.vector
                              cpeng.tensor_copy(
                                  dt_[0:64, r_lo * SW + 1 : r_hi * SW],
                                  xt[0:64, r_lo * SW : r_hi * SW - 1],
                              )

                      if "nomm" in ablate:
                          xflat = xt[:, : U * 224].rearrange("p (r c) -> p r c", c=224)
                          nc.sync.dma_start(o_ph[0, :, u0 : u0 + U, :], xflat)
                          nc.sync.dma_start(o_ph[1, :, u0 : u0 + U, :], xflat)
                          continue
                      obe = opool.tile([128, U, 224], BF16, name="obe")
                      obo = opool.tile([128, U, 224], BF16, name="obo")

                      if wmaj and "nomm" not in ablate:
                          assert use_c and use_d and "outpath" not in ablate
                          for j0 in range(0, U, 4):
                              s0a, s0b = j0 + 1, j0 + 3
                              pea = ppool.tile([128, 456], F32, name="pea")
                              poa = ppool.tile([128, 456], F32, name="poa")
                              peb = ppool.tile([128, 456], F32, name="peb")
                              pob = ppool.tile([128, 456], F32, name="pob")
                              pairs = ((s0a, pea, poa), (s0b, peb, pob))
                              for kw in range(3):
                                  for s0, pe_, _ in pairs:
                                      nc.tensor.matmul(
                                          pe_[:], wsb[:, kw, :],
                                          xt[:, s0 * SW + kw : s0 * SW + kw + 456],
                                          start=(kw == 0), stop=False,
                                      )
                              for s0, pe_, _ in pairs:
                                  nc.tensor.matmul(
                                      pe_[:], wsb[:, 6, :],
                                      ct[:, (s0 - 1) * SW + 1 : (s0 - 1) * SW + 457],
                                      start=False, stop=False,
                                  )
                              for s0, pe_, _ in pairs:
                                  nc.tensor.matmul(
                                      pe_[:], wsb[64:128, 10, :],
                                      xt[64:128, (s0 - 1) * SW + 2 : (s0 - 1) * SW + 458],
                                      start=False, stop=True,
                                  )
                              for kw in range(3):
                                  for s0, _, po_ in pairs:
                                      nc.tensor.matmul(
                                          po_[:], wsb[:, 3 + kw, :],
                                          xt[:, s0 * SW + kw : s0 * SW + kw + 456],
                                          start=(kw == 0), stop=False,
                                      )
                              for s0, _, po_ in pairs:
                                  nc.tensor.matmul(
                                      po_[:], wsb[:, 7, :],
                                      dt_[:, (s0 + 1) * SW + 1 : (s0 + 1) * SW + 457],
                                      start=False, stop=False,
                                  )
                              for s0, _, po_ in pairs:
                                  nc.tensor.matmul(
                                      po_[:], wsb[0:64, 10, :],
                                      xt[0:64, (s0 + 1) * SW + 2 : (s0 + 1) * SW + 458],
                                      start=False, stop=True,
                                  )
                              for jj, (s0, pe_, po_) in zip((j0, j0 + 2), pairs):
                                  pev = pe_[:].rearrange("p (r c) -> p r c", c=SW)
                                  pov = po_[:].rearrange("p (r c) -> p r c", c=SW)
                                  if act_mode == "dve":
                                      nc.vector.tensor_scalar_add(
                                          obe[:, jj : jj + 2, :], pev[:, :, 0:224], bias_sb[:]
                                      )
                                  else:
                                      nc.scalar.activation(
                                          obe[:, jj : jj + 2, :], pev[:, :, 0:224],
                                          mybir.ActivationFunctionType.Identity, bias=bias_sb[:],
                                      )
                                  if act_split:
                                      nc.vector.tensor_scalar_add(
                                          obo[:, jj : jj + 2, :], pov[:, :, 0:224], bias_sb[:]
                                      )
                                  else:
                                      nc.scalar.activation(
                                          obo[:, jj : jj + 2, :], pov[:, :, 0:224],
                                          mybir.ActivationFunctionType.Identity, bias=bias_sb[:],
                                      )
                              if st_split and "stores" not in ablate and j0 + 4 == U // 2:
                                  sth = nc.scalar if st_eng == "act" else nc.sync
                                  sth.dma_start(o_ph[0, :, u0 : u0 + U // 2, :], obe[:, 0 : U // 2, :])
                                  sth.dma_start(o_ph[1, :, u0 : u0 + U // 2, :], obo[:, 0 : U // 2, :])
                      for j0 in range(0, 0 if wmaj else U, 2):
                          if "nomm" in ablate:
                              continue
                          s0 = j0 + 1
                          pe = ppool.tile([128, 456], F32, name="pe")
                          po = ppool.tile([128, 456], F32, name="po")
                          for kw in range(3):
                              off = s0 * SW + kw
                              nc.tensor.matmul(
                                  pe[:], wsb[:, kw, :], xt[:, off : off + 456],
                                  start=(kw == 0), stop=False,
                              )
                          if use_c:
                              nc.tensor.matmul(
                                  pe[:], wsb[:, 6, :], ct[:, (s0 - 1) * SW + 1 : (s0 - 1) * SW + 457],
                                  start=False, stop=False,
                              )
                              nc.tensor.matmul(
                                  pe[:], wsb[64:128, 10, :],
                                  xt[64:128, (s0 - 1) * SW + 2 : (s0 - 1) * SW + 458],
                                  start=False, stop=True,
                              )
                          else:
                              for kw in range(3):
                                  nc.tensor.matmul(
                                      pe[:], wsb[64:128, 8 + kw, :],
                                      xt[64:128, (s0 - 1) * SW + kw : (s0 - 1) * SW + kw + 456],
                                      start=False, stop=(kw == 2),
                                  )
                          for kw in range(3):
                              off = s0 * SW + kw
                              nc.tensor.matmul(
                                  po[:], wsb[:, 3 + kw, :], xt[:, off : off + 456],
                                  start=(kw == 0), stop=False,
                              )
                          if use_d:
                              nc.tensor.matmul(
                                  po[:], wsb[:, 7, :], dt_[:, (s0 + 1) * SW + 1 : (s0 + 1) * SW + 457],
                                  start=False, stop=False,
                              )
                              nc.tensor.matmul(
                                  po[:], wsb[0:64, 10, :],
                                  xt[0:64, (s0 + 1) * SW + 2 : (s0 + 1) * SW + 458],
                                  start=False, stop=True,
                              )
                          else:
                              for kw in range(3):
                                  nc.tensor.matmul(
                                      po[:], wsb[0:64, 8 + kw, :],
                                      xt[0:64, (s0 + 1) * SW + kw : (s0 + 1) * SW + kw + 456],
                                      start=False, stop=(kw == 2),
                                  )
                          if "outpath" in ablate:
                              continue
                          pev = pe[:].rearrange("p (r c) -> p r c", c=SW)
                          pov = po[:].rearrange("p (r c) -> p r c", c=SW)
                          nc.scalar.activation(
                              obe[:, j0 : j0 + 2, :], pev[:, :, 0:224],
                              mybir.ActivationFunctionType.Identity, bias=bias_sb[:],
                          )
                          if act_split:
                              nc.vector.tensor_scalar_add(
                                  obo[:, j0 : j0 + 2, :], pov[:, :, 0:224], bias_sb[:]
                              )
                          else:
                              nc.scalar.activation(
                                  obo[:, j0 : j0 + 2, :], pov[:, :, 0:224],
                                  mybir.ActivationFunctionType.Identity, bias=bias_sb[:],
                              )

                      if "stores" not in ablate and "nomm" not in ablate:
                          st = nc.scalar if st_eng == "act" else nc.sync
                          if st_split and wmaj:
                              st.dma_start(o_ph[0, :, u0 + U // 2 : u0 + U, :], obe[:, U // 2 : U, :])
                              st.dma_start(o_ph[1, :, u0 + U // 2 : u0 + U, :], obo[:, U // 2 : U, :])
                          else:
                              st.dma_start(o_ph[0, :, u0 : u0 + U, :], obe[:])
                              st.dma_start(o_ph[1, :, u0 : u0 + U, :], obo[:])

    nc.finalize()
    return nc


def _pack_weights(weight):
    # wt[p, k, co]; k stacks (lower p<64 / upper p>=64):
    #   0..2  A-even (pe):  [w(kh=1,kw) ; w(kh=2,kw)]
    #   3..5  A-odd  (po):  [w(kh=0,kw) ; w(kh=1,kw)]
    #   6     C (pe):       [w(0,1) ; w(0,0)]   (upper reads shifted rows)
    #   7     D (po):       [w(2,0) ; w(2,1)]
    #   8..10 singles kw:   [w(2,kw) ; w(0,kw)]  (po lower / pe upper)
    wt = np.empty((128, 11, 128), dtype=np.float32)
    wT = {(kh, kw): weight[:, :, kh, kw].T for kh in range(3) for kw in range(3)}
    for kw in range(3):
        wt[:64, kw] = wT[(1, kw)]
        wt[64:, kw] = wT[(2, kw)]
        wt[:64, 3 + kw] = wT[(0, kw)]
        wt[64:, 3 + kw] = wT[(1, kw)]
        wt[:64, 8 + kw] = wT[(2, kw)]
        wt[64:, 8 + kw] = wT[(0, kw)]
    wt[:64, 6] = wT[(0, 1)]
    wt[64:, 6] = wT[(0, 0)]
    wt[:64, 7] = wT[(2, 0)]
    wt[64:, 7] = wT[(2, 1)]
    return wt.astype(NPBF16)


def _make_in_maps(x, weight, bias):
    x = np.asarray(x, dtype=np.float32)
    weight = np.asarray(weight, dtype=np.float32)
    bias = np.ascontiguousarray(np.asarray(bias, dtype=np.float32))
    wt = np.ascontiguousarray(_pack_weights(weight))

    xb = x.astype(NPBF16).reshape(N_FULL, C_IN, HH, 2, W)
    x_even = xb[:, :, :, 0, :]
    x_odd = xb[:, :, :, 1, :]

    xp = np.zeros((N_FULL, 2, C_IN, HH, SW), dtype=NPBF16)
    xp[:, 0, :, :, 1:225] = x_even
    xp[:, 1, :, :, 1:225] = x_odd

    return [
        {
            "xp": np.ascontiguousarray(xp[c * N_PER : (c + 1) * N_PER]),
            "wt": wt,
            "bias": bias,
        }
        for c in range(N_CORES)
    ]


def kernel(x, weight, bias, _trace=False):
    if "nc" not in _CACHE:
        _CACHE["nc"] = _build_nc()
    nc = _CACHE["nc"]

    in_maps = _make_in_maps(x, weight, bias)
    res = run_bass_kernel_spmd(
        nc, in_maps, core_ids=list(range(N_CORES)), trace=_trace
    )
    # out[n, ph, co, r, w] (bf16) -> full fp32 [N, co, 2r+ph, w]
    out = np.concatenate([r["out"] for r in res.results], axis=0)
    out = out.astype(np.float32).transpose(0, 2, 3, 1, 4).reshape(N_FULL, C_OUT, H, W)
    if _trace:
        _CACHE["last_result"] = res
    return np.ascontiguousarray(out)


if __name__ == "__main__":
    # quick numpy self-check of the tap algebra on a tiny random case
    rng = np.random.default_rng(0)
    x = rng.standard_normal((N_FULL, C_IN, H, W)).astype(np.float32)
    print("built in_maps ok:", len(_make_in_maps(x, rng.standard_normal((C_OUT, C_IN, 3, 3)).astype(np.float32), np.zeros(C_OUT, np.float32))))



# revision 23
# speedup vs baseline: 1.8783x; 1.8783x over previous
"""Conv2d 3x3 (stride 1, pad 1) forward, data-parallel over batch on 8 trn2 cores.

x (16,64,224,224) * w (128,64,3,3) + b (128,) -> (16,128,224,224), fp32.

Per core (2 images), compute in bf16 with fp32 PSUM accumulate:

- Rows are even/odd phase-split on the SBUF partition axis (p = phase*64 + ci).
  Host pre-packs x into a padded bf16 layout (xp) so every DMA moves >=6KB
  contiguous runs per partition (<512B chunks run at half DMA rate).
- Per PSUM tile (2 output rows x 228 = 456 wide) the 9 conv taps are covered
  by 5 matmuls (the minimum: 9 taps = 4 K=128 pairs + 1 K=64 single):
  3 "A" pairs (two kh taps at the same kw, from the xt tile), 1 "C"/"D" pair
  over a tile holding the remaining kh row at two horizontal shifts (two kw
  taps in one matmul), and 1 K=64 single. The shifted tiles are built from a
  6.5MB/img HBM re-read (the cross-partition half) plus an on-chip DVE
  column-shift copy (the same-partition half), instead of a full host-side
  duplicate.
- Matmuls are issued weight-major over pairs of j0 steps (KWMAJ): consecutive
  matmuls share the same stationary operand, halving PE weight reloads
  (measured ~17us/iter on hw). Output stores issue from the Activation
  engine's DGE queue so input loads (SP queue) are never stuck behind
  compute-dependent stores; obo bias-adds run on DVE to halve Act load.
- Outputs are written as bf16 in a phase-split layout; the host re-interleaves
  and upcasts to fp32. End-to-end rel err ~3.6e-3 (tolerance 2e-2).

Measured (1024-iter hw loop, slope method): ~310us/iter vs 557us baseline.
"""

import os
import sys

sys.path.insert(0, "/opt/trn_rl_repo")

import numpy as np
import ml_dtypes

import concourse.bass as bass
import concourse.mybir as mybir
import concourse.tile as tile
from concourse import bacc
from concourse.bass_utils import run_bass_kernel_spmd

N_FULL, C_IN, H, W = 16, 64, 224, 224
C_OUT = 128
N_CORES = 8
N_PER = N_FULL // N_CORES  # 2 images per core
HH = H // 2  # 112 row-pairs

U = int(os.environ.get("KU", "16"))  # row-pairs of output per block
SLOTS = U + 2
SW = int(os.environ.get("KSW", "226"))  # padded row width: [pad, 224 data, pad]
MW = 2 * SW  # matmul window width (2 row slots)
NB = HH // U  # 8 blocks per image
F32 = mybir.dt.float32
BF16 = mybir.dt.bfloat16
NPBF16 = ml_dtypes.bfloat16

_CACHE = {}


def _build_nc(repeat=1, loop_reps=0, ablate=(), scheme=None, xcd_bufs=None,
              out_bufs=None, psum_bufs=None):
    """scheme: 'dual' = C+D pairs (10 matmuls/j0), 'pe_only' = C pair only
    (11 matmuls), 'base' = no shifted tiles (12 matmuls)."""
    ablate = set(ablate) | {a for a in os.environ.get("KABLATE", "").split(",") if a}
    scheme = scheme or os.environ.get("KSCHEME", "dual")
    xcd_bufs = xcd_bufs or int(os.environ.get("KXCD", "3"))
    out_bufs = out_bufs or int(os.environ.get("KOUT", "3"))
    psum_bufs = psum_bufs or int(os.environ.get("KPSUM", "2"))
    st_eng = os.environ.get("KSTQ", "act")
    st2_eng = os.environ.get("KSTQ2", "act")  # queue for obo stores
    alt_sgl = os.environ.get("KALT", "1") == "1"
    xq = os.environ.get("KXQ", "sp,sp").split(",")  # xt load queue per phase
    c_src = os.environ.get("KCSRC", "hbm")
    act_mode = os.environ.get("KACT", "split")
    act_split = act_mode in ("split", "dve")
    wmaj = os.environ.get("KWMAJ", "1") == "1"
    ldq = os.environ.get("KLDQ", "sp")  # queue for ct/dt loads
    cpe_pool = os.environ.get("KCPE", "dve") == "pool"
    st_split = os.environ.get("KSTSPL", "0") == "1"
    xmrg = os.environ.get("KXMRG", "0") == "1"
    use_c = scheme in ("dual", "pe_only")
    use_d = scheme == "dual"
    nc = bacc.Bacc("TRN2", target_bir_lowering=False, debug=False)
    # Host-packed inputs (see _make_in_maps):
    #   xp[n, ph, ci, r, c]: row 2r+ph, buffer col c = x[c-1] (cols 1..224 data)
    #   xc[n, p, r, c]: p<64 odd rows shifted (col c = x[c-2]); p>=64 odd rows
    #   xd[n, p, r, c]: same for even rows
    xp_t = nc.dram_tensor("xp", [N_PER, 2, C_IN, HH, SW], BF16, kind="ExternalInput")
    wt_t = nc.dram_tensor("wt", [128, 11, 128], BF16, kind="ExternalInput")
    b_t = nc.dram_tensor("bias", [C_OUT], F32, kind="ExternalInput")
    o_t = nc.dram_tensor("out", [N_PER, 2, C_OUT, HH, 224], BF16, kind="ExternalOutput")
    xp_ap = xp_t.ap()
    wt_ap, b_ap, o_ap = wt_t.ap(), b_t.ap(), o_t.ap()

    with tile.TileContext(nc) as tc:
        with (
            tc.tile_pool(name="const", bufs=1) as cpool,
            tc.tile_pool(name="outp", bufs=out_bufs) as opool,
            tc.tile_pool(name="psum", bufs=psum_bufs, space="PSUM") as ppool,
        ):
            # Weight/bias loads go on the Act DGE queue so the Sync queue's
            # head is free for the first x tile chunks.
            wsb = cpool.tile([128, 11, 128], BF16)
            nc.scalar.dma_start(wsb[:], wt_ap)
            bias_sb = cpool.tile([128, 1], F32)
            nc.scalar.dma_start(bias_sb[:], b_ap[:, None])

            xbufs = [cpool.tile([128, SLOTS * SW + 4], BF16, name=f"xb{i}") for i in range(xcd_bufs)]
            cbufs = [cpool.tile([128, SLOTS * SW + 4], BF16, name=f"cb{i}") for i in range(xcd_bufs)] if use_c else []
            dbufs = [cpool.tile([128, SLOTS * SW + 4], BF16, name=f"db{i}") for i in range(xcd_bufs)] if use_d else []
            if os.environ.get("KINITMS", "0") == "1":
                # One-time whole-buffer clears. Not needed for correctness:
                # every slot a matmul window can reach is either DMA-loaded
                # (host-packed pads included), edge-memset per block, or lands
                # only in ignored psum pad columns.
                for tl in xbufs + cbufs + dbufs:
                    nc.vector.memset(tl[:], 0.0)

            import contextlib
            loop_cm = tc.For_i(0, loop_reps, 1) if loop_reps else contextlib.nullcontext()
            with loop_cm:
             for rep in range(repeat):
              for n in range(N_PER):
                  o_ph = o_ap[n]
                  for b in range(NB):
                      u0 = b * U
                      bi = (rep * N_PER * NB + n * NB + b) % xcd_bufs
                      xt = xbufs[bi]
                      ct = cbufs[bi] if use_c else None
                      dt_ = dbufs[bi] if use_d else None
                      xtv = xt[:, : SLOTS * SW].rearrange("p (r c) -> p r c", c=SW)

                      if b == 0:
                          nc.vector.memset(xt[:, 0:SW], 0.0)
                          if use_c:
                              nc.vector.memset(ct[:, 0:SW], 0.0)
                      if b == NB - 1:
                          nc.vector.memset(xt[:, (U + 1) * SW : (U + 2) * SW], 0.0)
                          if use_d:
                              nc.vector.memset(dt_[:, (U + 1) * SW : (U + 2) * SW], 0.0)
                      r_lo = 1 if b == 0 else 0
                      r_hi = U + 1 if b == NB - 1 else U + 2
                      pa = u0 - 1 + r_lo  # first row-pair index fetched
                      pb = u0 - 1 + r_hi  # one past last
                      # First block of the kernel: chunk the loads so group 0's
                      # matmuls start after ~1/3 of the tile has landed instead
                      # of waiting for the full 2MB.
                      first_blk = rep == 0 and n == 0 and b == 0
                      chunks = [(r_lo, 7), (7, 12), (12, r_hi)] if first_blk else [(r_lo, r_hi)]
                      ctv = ct[:, : SLOTS * SW].rearrange("p (r c) -> p r c", c=SW) if use_c else None
                      dtv = dt_[:, : SLOTS * SW].rearrange("p (r c) -> p r c", c=SW) if use_d else None
                      ldeng = {"pool": nc.gpsimd, "dve": nc.vector, "sp": nc.sync}[ldq]
                      cpeng = nc.gpsimd if cpe_pool else nc.vector
                      for ca, cb in chunks:
                          sa, sb = pa + (ca - r_lo), pa + (cb - r_lo)
                          lo = ca * SW + 1 if ca == r_lo else ca * SW
                          if "noload" not in ablate:
                              if xmrg:
                                  nc.sync.dma_start(
                                      xtv[:, ca:cb, :],
                                      xp_ap[n].rearrange("ph ci r c -> (ph ci) r c")[:, sa:sb, :],
                                  )
                              else:
                                  for ph in range(2):
                                      xeng = {"pool": nc.gpsimd, "sp": nc.sync, "act": nc.scalar}[xq[ph]]
                                      xeng.dma_start(
                                          xtv[ph * 64 : (ph + 1) * 64, ca:cb, :],
                                          xp_ap[n, ph, :, sa:sb, :],
                                      )
                          if use_c:
                              if c_src == "sbuf":
                                  ldeng.dma_start(ctv[0:64, ca:cb, :], xtv[64:128, ca:cb, :])
                              else:
                                  ldeng.dma_start(ctv[0:64, ca:cb, :], xp_ap[n, 1, :, sa:sb, :])
                              if "nocopy" not in ablate:
                                  cpeng.tensor_copy(
                                      ct[64:128, lo : cb * SW],
                                      xt[64:128, lo - 1 : cb * SW - 1],
                                  )
                          if use_d:
                              if c_src == "sbuf":
                                  ldeng.dma_start(dtv[64:128, ca:cb, :], xtv[0:64, ca:cb, :])
                              else:
                                  ldeng.dma_start(dtv[64:128, ca:cb, :], xp_ap[n, 0, :, sa:sb, :])
                              if "nocopy" not in ablate:
                                  cpeng.tensor_copy(
                                      dt_[0:64, lo : cb * SW],
                                      xt[0:64, lo - 1 : cb * SW - 1],
                                  )

                      if "nomm" in ablate:
                          xflat = xt[:, : U * 224].rearrange("p (r c) -> p r c", c=224)
                          nc.sync.dma_start(o_ph[0, :, u0 : u0 + U, :], xflat)
                          nc.sync.dma_start(o_ph[1, :, u0 : u0 + U, :], xflat)
                          continue
                      obe = opool.tile([128, U, 224], BF16, name="obe")
                      obo = opool.tile([128, U, 224], BF16, name="obo")

                      last_blk = n == N_PER - 1 and b == NB - 1
                      if wmaj and "nomm" not in ablate:
                          assert use_c and use_d and "outpath" not in ablate
                          for j0 in range(0, U, 4):
                              s0a, s0b = j0 + 1, j0 + 3
                              pea = ppool.tile([128, MW], F32, name="pea")
                              poa = ppool.tile([128, MW], F32, name="poa")
                              peb = ppool.tile([128, MW], F32, name="peb")
                              pob = ppool.tile([128, MW], F32, name="pob")
                              pairs = ((s0a, pea, poa), (s0b, peb, pob))
                              # A full-array LDWEIGHTS stalls (~105ns) after
                              # row-tiled matmuls and vice versa. Alternating
                              # singles-last / singles-first groups makes
                              # adjacent groups' singles runs contiguous,
                              # halving those transitions.
                              sgl_first = alt_sgl and (j0 // 4) % 2 == 1

                              def emit_singles(first):
                                  # K=64 singles, interleaved hi/lo: consecutive
                                  # instructions target disjoint PE row groups
                                  # (tile_position (64,0) vs (0,0)), so each
                                  # hi/lo pair runs concurrently in the array.
                                  for s0, pe_, po_ in pairs:
                                      nc.tensor.matmul(
                                          pe_[:], wsb[64:128, 10, :],
                                          xt[64:128, (s0 - 1) * SW + 2 : (s0 - 1) * SW + MW + 2],
                                          start=first, stop=not first,
                                      )
                                      nc.tensor.matmul(
                                          po_[:], wsb[0:64, 10, :],
                                          xt[0:64, (s0 + 1) * SW + 2 : (s0 + 1) * SW + MW + 2],
                                          start=first, stop=not first,
                                      )

                              if sgl_first:
                                  emit_singles(True)
                              for kw in range(3):
                                  for s0, pe_, _ in pairs:
                                      nc.tensor.matmul(
                                          pe_[:], wsb[:, kw, :],
                                          xt[:, s0 * SW + kw : s0 * SW + kw + MW],
                                          start=(kw == 0) and not sgl_first, stop=False,
                                      )
                              for s0, pe_, _ in pairs:
                                  nc.tensor.matmul(
                                      pe_[:], wsb[:, 6, :],
                                      ct[:, (s0 - 1) * SW + 1 : (s0 - 1) * SW + MW + 1],
                                      start=False, stop=sgl_first,
                                  )
                              for kw in range(3):
                                  for s0, _, po_ in pairs:
                                      nc.tensor.matmul(
                                          po_[:], wsb[:, 3 + kw, :],
                                          xt[:, s0 * SW + kw : s0 * SW + kw + MW],
                                          start=(kw == 0) and not sgl_first, stop=False,
                                      )
                              for s0, _, po_ in pairs:
                                  nc.tensor.matmul(
                                      po_[:], wsb[:, 7, :],
                                      dt_[:, (s0 + 1) * SW + 1 : (s0 + 1) * SW + MW + 1],
                                      start=False, stop=sgl_first,
                                  )
                              if not sgl_first:
                                  emit_singles(False)
                              for jj, (s0, pe_, po_) in zip((j0, j0 + 2), pairs):
                                  pev = pe_[:].rearrange("p (r c) -> p r c", c=SW)
                                  pov = po_[:].rearrange("p (r c) -> p r c", c=SW)
                                  if act_mode == "dve":
                                      nc.vector.tensor_scalar_add(
                                          obe[:, jj : jj + 2, :], pev[:, :, 0:224], bias_sb[:]
                                      )
                                  else:
                                      nc.scalar.activation(
                                          obe[:, jj : jj + 2, :], pev[:, :, 0:224],
                                          mybir.ActivationFunctionType.Identity, bias=bias_sb[:],
                                      )
                                  if act_split:
                                      nc.vector.tensor_scalar_add(
                                          obo[:, jj : jj + 2, :], pov[:, :, 0:224], bias_sb[:]
                                      )
                                  else:
                                      nc.scalar.activation(
                                          obo[:, jj : jj + 2, :], pov[:, :, 0:224],
                                          mybir.ActivationFunctionType.Identity, bias=bias_sb[:],
                                      )
                              if last_blk and "stores" not in ablate:
                                  # Final block: store each j0 group as soon
                                  # as its bias-adds land, to shrink the
                                  # end-of-kernel drain tail.
                                  sth = nc.scalar if st_eng == "act" else nc.sync
                                  sth2 = {"pool": nc.gpsimd, "sp": nc.sync, "act": nc.scalar}[st2_eng]
                                  sth.dma_start(
                                      o_ph[0, :, u0 + j0 : u0 + j0 + 4, :],
                                      obe[:, j0 : j0 + 4, :],
                                  )
                                  sth2.dma_start(
                                      o_ph[1, :, u0 + j0 : u0 + j0 + 4, :],
                                      obo[:, j0 : j0 + 4, :],
                                  )
                      for j0 in range(0, 0 if wmaj else U, 2):
                          if "nomm" in ablate:
                              continue
                          s0 = j0 + 1
                          pe = ppool.tile([128, MW], F32, name="pe")
                          po = ppool.tile([128, MW], F32, name="po")
                          for kw in range(3):
                              off = s0 * SW + kw
                              nc.tensor.matmul(
                                  pe[:], wsb[:, kw, :], xt[:, off : off + MW],
                                  start=(kw == 0), stop=False,
                              )
                          if use_c:
                              nc.tensor.matmul(
                                  pe[:], wsb[:, 6, :], ct[:, (s0 - 1) * SW + 1 : (s0 - 1) * SW + MW + 1],
                                  start=False, stop=False,
                              )
                              nc.tensor.matmul(
                                  pe[:], wsb[64:128, 10, :],
                                  xt[64:128, (s0 - 1) * SW + 2 : (s0 - 1) * SW + MW + 2],
                                  start=False, stop=True,
                              )
                          else:
                              for kw in range(3):
                                  nc.tensor.matmul(
                                      pe[:], wsb[64:128, 8 + kw, :],
                                      xt[64:128, (s0 - 1) * SW + kw : (s0 - 1) * SW + kw + MW],
                                      start=False, stop=(kw == 2),
                                  )
                          for kw in range(3):
                              off = s0 * SW + kw
                              nc.tensor.matmul(
                                  po[:], wsb[:, 3 + kw, :], xt[:, off : off + MW],
                                  start=(kw == 0), stop=False,
                              )
                          if use_d:
                              nc.tensor.matmul(
                                  po[:], wsb[:, 7, :], dt_[:, (s0 + 1) * SW + 1 : (s0 + 1) * SW + MW + 1],
                                  start=False, stop=False,
                              )
                              nc.tensor.matmul(
                                  po[:], wsb[0:64, 10, :],
                                  xt[0:64, (s0 + 1) * SW + 2 : (s0 + 1) * SW + MW + 2],
                                  start=False, stop=True,
                              )
                          else:
                              for kw in range(3):
                                  nc.tensor.matmul(
                                      po[:], wsb[0:64, 8 + kw, :],
                                      xt[0:64, (s0 + 1) * SW + kw : (s0 + 1) * SW + kw + MW],
                                      start=False, stop=(kw == 2),
                                  )
                          if "outpath" in ablate:
                              continue
                          pev = pe[:].rearrange("p (r c) -> p r c", c=SW)
                          pov = po[:].rearrange("p (r c) -> p r c", c=SW)
                          nc.scalar.activation(
                              obe[:, j0 : j0 + 2, :], pev[:, :, 0:224],
                              mybir.ActivationFunctionType.Identity, bias=bias_sb[:],
                          )
                          if act_split:
                              nc.vector.tensor_scalar_add(
                                  obo[:, j0 : j0 + 2, :], pov[:, :, 0:224], bias_sb[:]
                              )
                          else:
                              nc.scalar.activation(
                                  obo[:, j0 : j0 + 2, :], pov[:, :, 0:224],
                                  mybir.ActivationFunctionType.Identity, bias=bias_sb[:],
                              )

                      if "stores" not in ablate and "nomm" not in ablate and not (last_blk and wmaj):
                          st = nc.scalar if st_eng == "act" else nc.sync
                          st2 = {"pool": nc.gpsimd, "sp": nc.sync, "act": nc.scalar}[st2_eng]
                          if st_split:
                              st.dma_start(o_ph[0, :, u0 + U // 2 : u0 + U, :], obe[:, U // 2 : U, :])
                              st2.dma_start(o_ph[1, :, u0 + U // 2 : u0 + U, :], obo[:, U // 2 : U, :])
                          else:
                              st.dma_start(o_ph[0, :, u0 : u0 + U, :], obe[:])
                              st2.dma_start(o_ph[1, :, u0 : u0 + U, :], obo[:])

    nc.finalize()
    return nc


def _pack_weights(weight):
    # wt[p, k, co]; k stacks (lower p<64 / upper p>=64):
    #   0..2  A-even (pe):  [w(kh=1,kw) ; w(kh=2,kw)]
    #   3..5  A-odd  (po):  [w(kh=0,kw) ; w(kh=1,kw)]
    #   6     C (pe):       [w(0,1) ; w(0,0)]   (upper reads shifted rows)
    #   7     D (po):       [w(2,0) ; w(2,1)]
    #   8..10 singles kw:   [w(2,kw) ; w(0,kw)]  (po lower / pe upper)
    wt = np.empty((128, 11, 128), dtype=np.float32)
    wT = {(kh, kw): weight[:, :, kh, kw].T for kh in range(3) for kw in range(3)}
    for kw in range(3):
        wt[:64, kw] = wT[(1, kw)]
        wt[64:, kw] = wT[(2, kw)]
        wt[:64, 3 + kw] = wT[(0, kw)]
        wt[64:, 3 + kw] = wT[(1, kw)]
        wt[:64, 8 + kw] = wT[(2, kw)]
        wt[64:, 8 + kw] = wT[(0, kw)]
    wt[:64, 6] = wT[(0, 1)]
    wt[64:, 6] = wT[(0, 0)]
    wt[:64, 7] = wT[(2, 0)]
    wt[64:, 7] = wT[(2, 1)]
    return wt.astype(NPBF16)


def _make_in_maps(x, weight, bias):
    x = np.asarray(x, dtype=np.float32)
    weight = np.asarray(weight, dtype=np.float32)
    bias = np.ascontiguousarray(np.asarray(bias, dtype=np.float32))
    wt = np.ascontiguousarray(_pack_weights(weight))

    xb = x.astype(NPBF16).reshape(N_FULL, C_IN, HH, 2, W)
    x_even = xb[:, :, :, 0, :]
    x_odd = xb[:, :, :, 1, :]

    xp = np.zeros((N_FULL, 2, C_IN, HH, SW), dtype=NPBF16)
    xp[:, 0, :, :, 1:225] = x_even
    xp[:, 1, :, :, 1:225] = x_odd

    return [
        {
            "xp": np.ascontiguousarray(xp[c * N_PER : (c + 1) * N_PER]),
            "wt": wt,
            "bias": bias,
        }
        for c in range(N_CORES)
    ]


def kernel(x, weight, bias, _trace=False):
    if "nc" not in _CACHE:
        _CACHE["nc"] = _build_nc()
    nc = _CACHE["nc"]

    in_maps = _make_in_maps(x, weight, bias)
    res = run_bass_kernel_spmd(
        nc, in_maps, core_ids=list(range(N_CORES)), trace=_trace
    )
    # out[n, ph, co, r, w] (bf16) -> full fp32 [N, co, 2r+ph, w]
    out = np.concatenate([r["out"] for r in res.results], axis=0)
    out = out.astype(np.float32).transpose(0, 2, 3, 1, 4).reshape(N_FULL, C_OUT, H, W)
    if _trace:
        _CACHE["last_result"] = res
    return np.ascontiguousarray(out)


if __name__ == "__main__":
    # quick numpy self-check of the tap algebra on a tiny random case
    rng = np.random.default_rng(0)
    x = rng.standard_normal((N_FULL, C_IN, H, W)).astype(np.float32)
    print("built in_maps ok:", len(_make_in_maps(x, rng.standard_normal((C_OUT, C_IN, 3, 3)).astype(np.float32), np.zeros(C_OUT, np.float32))))

